# revision 3
# baseline (speedup 1.0000x reference)
"""Trainium2 Bass kernel for nn_ContrastiveLoss_V4.

Math: loss = (pos_loss + neg_loss) / n_comparisons over N=16384 L2-normalized
D=64 embeddings with C=128 labels.

Device computes the O(N^2) part: neg_loss = sum over different-label ordered
pairs of relu(1 - dist_ij)^2.  Key transformations:
  * d2_ij = sq_i + sq_j - 2 e_i.e_j (+eps terms) is produced by ONE matmul of
    augmented vectors: u=[e, 1.375, 1, 1], v=[-2e, 1.375, 0.109375, ~1e-6]
    (constants chosen exactly representable in bf16 so 1.375^2+0.109375 = 2.0
    == sq_i+sq_j up to fp32 normalization noise ~5e-7).
  * The label mask is folded into the same matmul: +16*one_hot(lab_i).one_hot(lab_j)
    pushes same-label pairs' d2 to >=16 so dist>=4 and the hinge is exactly 0.
  * d2 is (anti)symmetric up to O(eps): compute only supertiles (a,b), a<=b, of
    the 16x16 grid of 1024x1024 blocks and weight off-diagonal blocks by 2.
  * Per 128x512 psum tile pipeline: PE matmul (bf16, 2 K-chunks) -> ACT Sqrt
    (PSUM->SBUF bf16) -> DVE tensor_scalar y=(min(dist,1)-1) -> DVE
    scalar_tensor_tensor dump=y*y with accum_out giving the per-row sums.
pos_loss (O(N*D)), the comparison count (O(N)) and the final combine are host-side.
"""

import sys

sys.path.insert(0, "/opt/trn_rl_repo")

import numpy as np
import ml_dtypes

import concourse.bass as bass
import concourse.tile as tile
from concourse import bacc, mybir
from concourse.bass_utils import run_bass_kernel_spmd

N, D, C = 16384, 64, 128
MARGIN = 1.0
EPS_NORM = 1e-6
EPS_PD = 1e-6
B_MASK = 16.0          # added to d2 for same-label pairs; any value > (1+eps)^2 works
C_GUARD = 1e-6         # keeps sqrt input strictly positive for pathological inputs

N_CORES = 8
SUPER = 1024           # supertile edge
G = N // SUPER         # 16x16 supertile grid
KA = 67                # chunk-a rows: 64 embedding dims + 3 constant rows
KB = C                 # chunk-b rows: one-hot labels

BF = mybir.dt.bfloat16
F32 = mybir.dt.float32


def _work_assignment():
    """Triangle supertiles (a,b), a<=b, packed into per-core items.

    Returns per-core list of items; item = (a, [b1, b2], weight) with weight 2
    for off-diagonal supertiles, 1 for diagonal. Every core gets 7 two-panel
    items and 3 one-panel items (56 pairs + 24 singles total).
    """
    pairs, singles = [], []
    for a in range(G):
        offs = list(range(a + 1, G))
        while len(offs) >= 2:
            pairs.append((a, [offs.pop(0), offs.pop(0)], 2.0))
        for b in offs:
            singles.append((a, [b], 2.0))
        singles.append((a, [a], 1.0))
    assert len(pairs) == 7 * N_CORES and len(singles) == 3 * N_CORES
    cores = []
    for k in range(N_CORES):
        cores.append(pairs[k::N_CORES] + singles[k::N_CORES])
    return cores


_ASSIGN = _work_assignment()
N_ITEMS = 10
U_COLS = N_ITEMS * SUPER            # 10240
V_COLS = (7 * 2 + 3) * SUPER        # 17408
ACC_COLS = N_ITEMS * 8              # 80

_compiled = None


def _build_program(repeat=1):
    nc = bacc.Bacc("TRN2", target_bir_lowering=False, debug=False,
                   num_devices=N_CORES)
    ua = nc.dram_tensor("ua", [KA, U_COLS], BF, kind="ExternalInput").ap()
    ub = nc.dram_tensor("ub", [KB, U_COLS], BF, kind="ExternalInput").ap()
    va = nc.dram_tensor("va", [KA, V_COLS], BF, kind="ExternalInput").ap()
    vb = nc.dram_tensor("vb", [KB, V_COLS], BF, kind="ExternalInput").ap()
    acc_d = nc.dram_tensor("acc", [128, ACC_COLS], F32, kind="ExternalOutput").ap()

    with tile.TileContext(nc) as tc:
        with (
            tc.tile_pool(name="upool", bufs=2) as upool,
            tc.tile_pool(name="vpool", bufs=2) as vpool,
            tc.tile_pool(name="work", bufs=3) as work,
            tc.tile_pool(name="accp", bufs=1) as accp,
            tc.tile_pool(name="psum", bufs=2, space=bass.MemorySpace.PSUM) as psum,
        ):
            import contextlib
            loop_cm = tc.For_i(0, repeat) if repeat > 1 else contextlib.nullcontext()
            with loop_cm:
                _emit_body(nc, upool, vpool, work, accp, psum,
                           ua, ub, va, vb, acc_d)
    nc.compile()
    return nc


def _emit_body(nc, upool, vpool, work, accp, psum, ua, ub, va, vb, acc_d):
            acc = accp.tile([128, ACC_COLS], F32)
            v_off = 0
            for it in range(N_ITEMS):
                W = 2048 if it < 7 else 1024
                ua_t = upool.tile([KA, SUPER], BF, tag="ua")
                nc.sync.dma_start(ua_t[:], ua[:, it * SUPER:(it + 1) * SUPER])
                ub_t = upool.tile([KB, SUPER], BF, tag="ub")
                nc.sync.dma_start(ub_t[:], ub[:, it * SUPER:(it + 1) * SUPER])
                va_t = vpool.tile([KA, 2048], BF, tag="va")
                nc.sync.dma_start(va_t[:, :W], va[:, v_off:v_off + W])
                vb_t = vpool.tile([KB, 2048], BF, tag="vb")
                nc.sync.dma_start(vb_t[:, :W], vb[:, v_off:v_off + W])
                v_off += W

                for rb in range(8):
                    ps = psum.tile([128, 2048], F32, tag="ps")
                    lhs_a = ua_t[:, rb * 128:(rb + 1) * 128]
                    lhs_b = ub_t[:, rb * 128:(rb + 1) * 128]
                    for c in range(0, W, 512):
                        nc.tensor.matmul(ps[:, c:c + 512], lhs_a,
                                         va_t[:, c:c + 512], start=True, stop=False)
                    for c in range(0, W, 512):
                        nc.tensor.matmul(ps[:, c:c + 512], lhs_b,
                                         vb_t[:, c:c + 512], start=False, stop=True)
                    dist = work.tile([128, 2048], BF, tag="dist")
                    nc.scalar.activation(dist[:, :W], ps[:, :W],
                                         mybir.ActivationFunctionType.Sqrt)
                    yt = work.tile([128, 2048], BF, tag="y")
                    nc.vector.tensor_scalar(yt[:, :W], dist[:, :W], 1.0, 1.0,
                                            mybir.AluOpType.min,
                                            mybir.AluOpType.subtract)
                    dump = work.tile([128, 2048], BF, tag="dump")
                    nc.vector.scalar_tensor_tensor(
                        dump[:, :W], yt[:, :W], 0.0, yt[:, :W],
                        mybir.AluOpType.add, mybir.AluOpType.mult,
                        accum_out=acc[:, it * 8 + rb: it * 8 + rb + 1])
            nc.sync.dma_start(acc_d[:], acc[:])


def _prepare_inputs(embeddings):
    e = embeddings.astype(np.float32)
    nrm = np.linalg.norm(e, axis=1, keepdims=True)
    e = e / np.maximum(nrm, EPS_NORM)
    return e


def _make_in_maps(e, lab):
    # ---- build augmented operand matrices (bf16) ----
    ebf = e.astype(ml_dtypes.bfloat16)
    Ua = np.empty((KA, N), dtype=ml_dtypes.bfloat16)
    Va = np.empty((KA, N), dtype=ml_dtypes.bfloat16)
    Ua[:D] = ebf.T
    Va[:D] = (-2.0 * ebf.astype(np.float32)).astype(ml_dtypes.bfloat16).T
    # constants: 1.375*1.375 + 1*0.109375 = 2.0 exactly; + tiny guard
    Ua[D] = 1.375
    Va[D] = 1.375
    Ua[D + 1] = 1.0
    Va[D + 1] = 0.109375
    Ua[D + 2] = 1.0
    Va[D + 2] = np.float32(D * EPS_PD * EPS_PD + C_GUARD)
    onehot = np.zeros((C, N), dtype=ml_dtypes.bfloat16)
    onehot[lab, np.arange(N)] = 1.0
    Ub = (onehot.astype(np.float32) * B_MASK).astype(ml_dtypes.bfloat16)
    Vb = onehot

    # ---- pack per-core triangle work items ----
    in_maps = []
    weights = []
    for k in range(N_CORES):
        items = _ASSIGN[k]
        ua_p = np.empty((KA, U_COLS), dtype=ml_dtypes.bfloat16)
        ub_p = np.empty((KB, U_COLS), dtype=ml_dtypes.bfloat16)
        va_p = np.empty((KA, V_COLS), dtype=ml_dtypes.bfloat16)
        vb_p = np.empty((KB, V_COLS), dtype=ml_dtypes.bfloat16)
        w_k = []
        v_off = 0
        for i, (a, bs, w) in enumerate(items):
            ua_p[:, i * SUPER:(i + 1) * SUPER] = Ua[:, a * SUPER:(a + 1) * SUPER]
            ub_p[:, i * SUPER:(i + 1) * SUPER] = Ub[:, a * SUPER:(a + 1) * SUPER]
            for b in bs:
                va_p[:, v_off:v_off + SUPER] = Va[:, b * SUPER:(b + 1) * SUPER]
                vb_p[:, v_off:v_off + SUPER] = Vb[:, b * SUPER:(b + 1) * SUPER]
                v_off += SUPER
            w_k.append(w)
        assert v_off == V_COLS
        weights.append(w_k)
        in_maps.append({"ua": ua_p, "ub": ub_p, "va": va_p, "vb": vb_p})
    return in_maps, weights


def kernel(embeddings, labels, pos_idx, _trace=False):
    global _compiled
    e = _prepare_inputs(embeddings)
    lab = labels[:, 0].astype(np.int64)
    pidx = pos_idx.astype(np.int64)

    # ---- host side (O(N*D)): pos_loss, denominator ----
    e64 = e.astype(np.float64)
    sq = (e64 * e64).sum(1)
    s = e64.sum(1)
    ep = e64[pidx]
    d2p = (sq + sq[pidx] - 2.0 * (e64 * ep).sum(1)
           + 2.0 * EPS_PD * (s - s[pidx]) + D * EPS_PD * EPS_PD)
    pos_loss = np.maximum(d2p, 0.0).sum()
    cnt = np.bincount(lab, minlength=C)
    n_comp = N + (N * N - int((cnt.astype(np.int64) ** 2).sum()))

    in_maps, weights = _make_in_maps(e, lab)

    # ---- compile (cached) and run on 8 cores ----
    if _compiled is None:
        _compiled = _build_program()
    res = run_bass_kernel_spmd(_compiled, in_maps, list(range(N_CORES)),
                               trace=_trace)
    if _trace:
        global _last_profile
        _last_profile = res

    # ---- combine: weighted sum of per-item accumulators ----
    neg_loss = 0.0
    for k in range(N_CORES):
        a = res.results[k]["acc"].astype(np.float64)   # [128, 80]
        per_item = a.reshape(128, N_ITEMS, 8).sum(axis=(0, 2))
        neg_loss += float((per_item * np.asarray(weights[k])).sum())

    total = (pos_loss + neg_loss) / float(n_comp)
    return np.float32(total)


if __name__ == "__main__":
    rng = np.random.default_rng(0)
    emb = rng.standard_normal((N, D)).astype(np.float32)
    labels = (np.arange(N) % C).astype(np.int32).reshape(N, 1)
    pos_idx = ((np.arange(N) + C) % N).astype(np.int32)
    out = kernel(embeddings=emb, labels=labels, pos_idx=pos_idx)
    print("kernel out:", out)

